# revision 1
# baseline (speedup 1.0000x reference)
"""Trainium2 Bass kernel for nn_Def_A2C_Sample_Generator.

Computation (see reference):
  x = concat(state, payoff, noise)            (500, 504)
  h1 = lrelu(bn(adj @ (x @ w1) + b1))         (500, 32)
  h2 = lrelu(bn(adj @ (h1 @ w2) + b2))        (500, 16)
  xf = h2.reshape(8000)
  logits = xf @ actgen_w + def_cur_loc @ actgen_v          (50, 500)
  out = softmax(logits[None] + gumbel(u), axis=-1)         (1000, 50, 500)

Sharding: data-parallel over the 1000 samples, 125 per core on 8
cores. Each core computes the logits redundantly (small GCN in f32;
the 16 MB actgen_w is streamed in bf16 -- validated 5e-5 output rel
err) and softmaxes its own 125 x 50 x 500 gumbel block.

Softmax is factored so every ACT pass is independent of the logits
(the logits path is the serial prologue; all gumbel work front-runs
it):
  exp(logits + g) with g = -ln(-ln u) equals L * a where
  L = exp(logits) (prologue, 50x500) and a = exp(-ln(-ln u)) = -1/ln u.
Main loop, 5-r chunks in the natural (sample, r, T) layout (each
partition streams 10KB contiguous HBM runs):
  a       : 3 chunk-wide in-place ACT passes (Ln, Ln(-x), Exp(-x);
            one table set - see the act-table monkeypatch below)
  L bcast : per-r PE ones-matmul, hi+lo bf16 planes into f32 PSUM
            (full-rate PE vs 4x slower f32 moving data)
  q, S    : DVE scalar_tensor_tensor mult with fused row-sum accum
  out     : DVE reciprocal(S) + tensor_scalar mult into the chunk
            output tile, one 1.25MB store per chunk
DMA queues: u loads on the sync HWDGE ring, late-needed params on the
scalar ring, actgen_w stream + output stores on the gpsimd SWDGE path.
Logits rows are packed into 3 lanes at base partitions 0/32/64 (the
only legal matmul operand bases) via a DRAM bounce.

Measured: ~259us/core HW exec, output rel err 8e-5 vs the f32
reference.
"""
import sys

if "/opt/trn_rl_repo" not in sys.path:
    sys.path.insert(0, "/opt/trn_rl_repo")

import numpy as np

import concourse.bacc as bacc
import concourse.bass as bass
import concourse.mybir as mybir
import concourse.tile as tile
from concourse import bass_utils

# The act-table-load pass resolves Exp -> exp_and_others (id 0) and
# Ln -> natural_log (id 5), thrashing a ~2.7us table swap at every
# Ln<->Exp transition in the main loop. natural_log_exp_and_others
# (id 6) holds BOTH. Hide exp/ln from the other sets in the map the
# chooser reads (ids keep indexing the real act_info.json, so the
# loaded tables are unchanged) so every Exp and Ln lands on set 6 and
# one load suffices.
_orig_get_act_tables = bacc.get_activation_tables


def _patched_get_act_tables(arch):
    tabs = dict(_orig_get_act_tables(arch))
    both = {mybir.ActivationFunctionType.Exp, mybir.ActivationFunctionType.Ln}
    for name, fns in tabs.items():
        if name != "natural_log_exp_and_others" and (both & fns):
            tabs[name] = fns - both
    return tabs


bacc.get_activation_tables = _patched_get_act_tables

F32 = mybir.dt.float32
BF16 = mybir.dt.bfloat16
NCORES = 8
T = 500
R = 50
NS = 1000
SP = NS // NCORES  # 125 samples per core
H1, H2 = 32, 16
FIN = 504  # 2 + 500 + 2 input features
KT = 4  # K/M tiling of the 500 dim into 4x125
G = 2  # r's processed per main-loop iteration
NEG_SLOPE = 0.2

Z_MODE = "replicated"  # "allreduce" (c-sharded actgen_w + collective) | "replicated" (bf16 streamed)

_CACHE = {}


def _build(z_mode):
    nc = bacc.Bacc("TRN2", target_bir_lowering=False, debug=False,
                   enable_asserts=False, num_devices=NCORES)

    # ---- I/O ----
    din = {}
    din["xT"] = nc.dram_tensor("xT", [FIN, T], F32, kind="ExternalInput")
    din["adjT"] = nc.dram_tensor("adjT", [T, T], F32, kind="ExternalInput")
    din["w1"] = nc.dram_tensor("w1", [FIN, H1], F32, kind="ExternalInput")
    din["b1"] = nc.dram_tensor("b1", [1, H1], F32, kind="ExternalInput")
    din["w2"] = nc.dram_tensor("w2", [H1, H2], F32, kind="ExternalInput")
    din["b2"] = nc.dram_tensor("b2", [1, H2], F32, kind="ExternalInput")
    din["grow"] = nc.dram_tensor("grow", [1, T], F32, kind="ExternalInput")
    din["brow"] = nc.dram_tensor("brow", [1, T], F32, kind="ExternalInput")
    din["dclT"] = nc.dram_tensor("dclT", [T, R], F32, kind="ExternalInput")
    din["av"] = nc.dram_tensor("av", [T, T], F32, kind="ExternalInput")
    n_c = 2 if z_mode == "allreduce" else H2
    wr_dt = F32 if z_mode == "allreduce" else BF16
    din["wr"] = nc.dram_tensor("wr", [n_c, T, T], wr_dt, kind="ExternalInput")
    din["ident"] = nc.dram_tensor("ident", [128, 128], F32, kind="ExternalInput")
    din["u"] = nc.dram_tensor("u", [SP, R, T], F32, kind="ExternalInput")
    out = nc.dram_tensor("out", [SP, R, T], F32, kind="ExternalOutput")

    with tile.TileContext(nc) as tc:
        _emit(nc, tc, din, out, z_mode)
    nc.compile()
    return nc


def _emit(nc, tc, din, out, z_mode):
    from contextlib import ExitStack

    ctx = ExitStack()
    with ctx:
        # ---------- pools ----------
        const = ctx.enter_context(tc.tile_pool(name="const", bufs=1))
        small = ctx.enter_context(tc.tile_pool(name="small", bufs=1))
        psum = ctx.enter_context(tc.tile_pool(name="psum", bufs=1, space="PSUM"))
        dram = ctx.enter_context(tc.tile_pool(name="dram", bufs=1, space="DRAM"))

        # ---------- main-loop pools (created first: the first u-chunk
        # loads go ahead of the params in the sync FIFO so the ACT
        # front-run starts immediately) ----------
        CH = 5  # r's per chunk
        CW = CH * T
        upool = ctx.enter_context(tc.tile_pool(name="upool", bufs=5))
        opool = ctx.enter_context(tc.tile_pool(name="opool", bufs=3))
        qpool = ctx.enter_context(tc.tile_pool(name="qpool", bufs=6))
        spool = ctx.enter_context(tc.tile_pool(name="spool", bufs=8))
        bppool = ctx.enter_context(tc.tile_pool(name="bppool", bufs=5,
                                                space="PSUM"))
        pre_ut = {}
        for r0 in (0, CH):
            ut = upool.tile([SP, CW], F32, tag="u", name="u")
            nc.sync.dma_start(
                ut[:].rearrange("p (c t) -> p c t", c=CH),
                din["u"][:, r0:r0 + CH, :])
            pre_ut[r0] = ut

        # ---------- load params ----------
        ident = const.tile([128, 128], F32, tag="ident", name="ident")
        nc.sync.dma_start(ident[:], din["ident"][:])
        ones = const.tile([65, 128], F32, tag="ones", name="ones")
        nc.vector.memset(ones[:], 1.0)

        xT = [const.tile([126, T], F32, tag=f"xT{k}", name=f"xT{k}") for k in range(KT)]
        for k in range(KT):
            nc.sync.dma_start(xT[k][:], din["xT"][k * 126:(k + 1) * 126, :])
        adjT = [const.tile([125, T], F32, tag=f"adjT{k}", name=f"adjT{k}") for k in range(KT)]
        for k in range(KT):
            nc.sync.dma_start(adjT[k][:], din["adjT"][k * 125:(k + 1) * 125, :])
        w1 = [const.tile([126, H1], F32, tag=f"w1{k}", name=f"w1{k}") for k in range(KT)]
        for k in range(KT):
            nc.sync.dma_start(w1[k][:], din["w1"][k * 126:(k + 1) * 126, :])
        b1 = const.tile([1, H1], F32, tag="b1", name="b1")
        nc.sync.dma_start(b1[:], din["b1"][:])
        w2 = const.tile([H1, H2], F32, tag="w2", name="w2")
        nc.sync.dma_start(w2[:], din["w2"][:])
        b2 = const.tile([1, H2], F32, tag="b2", name="b2")
        nc.sync.dma_start(b2[:], din["b2"][:])
        grow = const.tile([1, T], F32, tag="grow", name="grow")
        nc.sync.dma_start(grow[:], din["grow"][:])
        brow = const.tile([1, T], F32, tag="brow", name="brow")
        nc.sync.dma_start(brow[:], din["brow"][:])
        dclT = [const.tile([125, R], F32, tag=f"dclT{k}", name=f"dclT{k}") for k in range(KT)]
        for k in range(KT):
            nc.scalar.dma_start(dclT[k][:], din["dclT"][k * 125:(k + 1) * 125, :])
        av = [const.tile([125, T], F32, tag=f"av{k}", name=f"av{k}") for k in range(KT)]
        for k in range(KT):
            nc.scalar.dma_start(av[k][:], din["av"][k * 125:(k + 1) * 125, :])

        # ---------- GCN, transposed formulation ----------
        # bn is folded into the adjacency on the host (adjT ships
        # gamma[t]*adj[t,u] transposed), leaving rank-1 bias terms:
        #   bn(adj@xw+b)^T[c,t] = (xw^T adj1^T)[c,t] + b[c]*gamma[t]
        #                         + beta[t]
        # so each adj product is ONE [H,500] PSUM accumulation of 4
        # K-tiles plus two K=1 bias matmuls, and layer 2 consumes h1T
        # directly as its stationary operand (no transposes, no bn DVE
        # chain).
        def lrelu_from_psum(ps_ap, out_tile, width):
            tmp = small.tile([width, T], F32, tag=f"lr{width}", name=f"lr{width}")
            nc.vector.tensor_scalar_mul(tmp[:], ps_ap, NEG_SLOPE)
            nc.vector.tensor_tensor(out_tile[:], tmp[:], ps_ap,
                                    op=mybir.AluOpType.max)

        xw1 = [small.tile([125, H1], F32, tag=f"xw1{m}", name=f"xw1{m}") for m in range(KT)]
        for m in range(KT):
            ps = psum.tile([125, H1], F32, tag="ps_small", name="ps_small")
            for k in range(KT):
                nc.tensor.matmul(ps[:], xT[k][:, m * 125:(m + 1) * 125],
                                 w1[k][:], start=(k == 0), stop=(k == KT - 1))
            nc.vector.tensor_copy(xw1[m][:], ps[:])

        a1ps = psum.tile([H1, T], F32, tag="ps_small", name="ps_small")
        for k in range(KT):
            nc.tensor.matmul(a1ps[:], xw1[k][:], adjT[k][:],
                             start=(k == 0), stop=False)
        nc.tensor.matmul(a1ps[:], b1[:], grow[:], start=False, stop=False)
        nc.tensor.matmul(a1ps[:], ones[0:1, :H1], brow[:], start=False,
                         stop=True)
        h1T = small.tile([H1, T], F32, tag="h1T", name="h1T")
        lrelu_from_psum(a1ps[:], h1T, H1)

        xw2 = [small.tile([125, H2], F32, tag=f"xw2{m}", name=f"xw2{m}") for m in range(KT)]
        for m in range(KT):
            ps = psum.tile([125, H2], F32, tag="ps_small", name="ps_small")
            nc.tensor.matmul(ps[:], h1T[:, m * 125:(m + 1) * 125], w2[:],
                             start=True, stop=True)
            nc.vector.tensor_copy(xw2[m][:], ps[:])

        a2ps = psum.tile([H2, T], F32, tag="ps_small", name="ps_small")
        for k in range(KT):
            nc.tensor.matmul(a2ps[:], xw2[k][:], adjT[k][:],
                             start=(k == 0), stop=False)
        nc.tensor.matmul(a2ps[:], b2[:], grow[:], start=False, stop=False)
        nc.tensor.matmul(a2ps[:], ones[0:1, :H2], brow[:], start=False,
                         stop=True)
        h2T = small.tile([H2, T], F32, tag="h2T", name="h2T")
        lrelu_from_psum(a2ps[:], h2T, H2)

        # h2 back to [t, c] tiles in bf16 for the z matmuls
        h2b = [small.tile([125, H2], BF16, tag=f"h2b{k}", name=f"h2b{k}")
               for k in range(KT)]
        for k in range(KT):
            pt = psum.tile([125, H2], F32, tag="ps_small", name="ps_small")
            nc.tensor.transpose(pt[:], h2T[:, k * 125:(k + 1) * 125],
                                ident[:H2, :H2])
            nc.vector.tensor_copy(h2b[k][:], pt[:])

        # ---------- z = xf @ actgen_w  (c-sharded or replicated) ----------
        zps = psum.tile([1, T], F32, tag="ps_z", name="ps_z")
        if z_mode == "allreduce":
            raise NotImplementedError("allreduce z-path retired")
            n_c = 2
            wr = [[const.tile([125, T], F32, tag=f"wr{c}_{k}", name=f"wr{c}_{k}") for k in range(KT)]
                  for c in range(n_c)]
            for c in range(n_c):
                for k in range(KT):
                    nc.sync.dma_start(wr[c][k][:],
                                      din["wr"][c, k * 125:(k + 1) * 125, :])
            first = True
            for c in range(n_c):
                for k in range(KT):
                    nc.tensor.matmul(zps[:], h2[k][:, c:c + 1], wr[c][k][:],
                                     start=first,
                                     stop=(c == n_c - 1 and k == KT - 1))
                    first = False
            zpart = small.tile([1, T], F32, tag="zpart", name="zpart")
            nc.scalar.copy(zpart[:], zps[:])
            zin = dram.tile([1, T], F32, name="zin")
            zout = dram.tile([1, T], F32, name="zout")
            nc.sync.dma_start(zin[:], zpart[:])
            nc.gpsimd.collective_compute(
                "AllReduce", mybir.AluOpType.add,
                replica_groups=[list(range(NCORES))],
                ins=[zin.opt()], outs=[zout.opt()])
            zrow = small.tile([1, T], F32, tag="zrow", name="zrow")
            nc.sync.dma_start(zrow[:], zout[:])
        else:
            # stream the 8MB bf16 actgen_w; bf16 matmul with f32 PSUM
            # accumulation costs ~5e-5 output rel err (validated).
            # one 500KB DMA per channel (4 partition-blocks packed along
            # the free dim) -- 64 individual SWDGE DMAs serialize at ~1.5us
            # each on the Q7 descriptor path
            wpool = ctx.enter_context(tc.tile_pool(name="wpool", bufs=9))
            first = True
            for c in range(H2):
                wt = wpool.tile([125, KT * T], BF16, tag="wr_stream",
                                name="wr_stream")
                nc.gpsimd.dma_start(
                    wt[:].rearrange("p (k n) -> p k n", k=KT),
                    din["wr"][c].rearrange("(k p) n -> p k n", k=KT))
                for k in range(KT):
                    nc.tensor.matmul(zps[:], h2b[k][:, c:c + 1],
                                     wt[:, k * T:(k + 1) * T],
                                     start=first,
                                     stop=(c == H2 - 1 and k == KT - 1))
                    first = False
            zrow = small.tile([1, T], F32, tag="zrow", name="zrow")
            nc.vector.tensor_copy(zrow[:], zps[:])

        # ---------- logits = dcl @ av + z (broadcast over rows) ----------
        lgp = psum.tile([R, T], F32, tag="ps_lg", name="ps_lg")
        for k in range(KT):
            nc.tensor.matmul(lgp[:], dclT[k][:], av[k][:],
                             start=(k == 0), stop=False)
        nc.tensor.matmul(lgp[:], ones[0:1, :R], zrow[:], start=False, stop=True)
        # matmul operands need base partition in {0, 32, 64}; pack the 50
        # logits rows into 3 lanes at those partitions, 17 rows each along
        # the free dim (a flat [1, 25000] tile would cost 100KB/partition
        # of SBUF since allocations are column slabs). Bounce through DRAM
        # to reshape partitions->free in 3 DMAs.
        LPL = 17  # logits rows per lane
        # lgflat holds L = exp(logits): the softmax is computed as
        # out = a*L / sum(a*L) with a = exp(-ln(-ln u)) = -1/ln(u),
        # keeping every ACT pass independent of the logits. L is split
        # hi/lo into two bf16 planes so the per-r partition-broadcast
        # matmuls run at full PE rate (fp32 moving data is 4x slower)
        # while keeping ~16 mantissa bits.
        logits = small.tile([R, T], F32, tag="logits", name="logits")
        nc.scalar.activation(logits[:], lgp[:],
                             mybir.ActivationFunctionType.Exp)
        lghi = small.tile([R, T], BF16, tag="lghi", name="lghi")
        nc.vector.tensor_copy(lghi[:], logits[:])
        lglo32 = small.tile([R, T], F32, tag="lglo32", name="lglo32")
        nc.vector.tensor_tensor(lglo32[:], logits[:], lghi[:],
                                op=mybir.AluOpType.subtract)
        lglo = small.tile([R, T], BF16, tag="lglo", name="lglo")
        nc.vector.tensor_copy(lglo[:], lglo32[:])
        onesb = const.tile([65, 128], BF16, tag="onesb", name="onesb")
        nc.vector.memset(onesb[:], 1.0)

        lgflat = {}
        for nm, src in (("hi", lghi), ("lo", lglo)):
            ld = dram.tile([R, T], BF16, name=f"ldram_{nm}")
            nc.sync.dma_start(ld[:], src[:])
            fl = small.tile([65, LPL * T], BF16, tag=f"lgflat{nm}",
                            name=f"lgflat{nm}")
            nc.sync.dma_start(
                fl[0:33:32, :].rearrange("l (j t) -> l j t", j=LPL),
                ld[0:2 * LPL].rearrange("(l j) t -> l j t", l=2))
            nc.sync.dma_start(fl[64:65, :(R - 2 * LPL) * T],
                              ld[2 * LPL:R].rearrange("(o j) t -> o (j t)", o=1))
            lgflat[nm] = fl

        def lg_slice(r):
            lane, j = r // LPL, r % LPL
            sl = (slice(lane * 32, lane * 32 + 1), slice(j * T, (j + 1) * T))
            return (lgflat["hi"][sl[0], sl[1]], lgflat["lo"][sl[0], sl[1]],
                    onesb[lane * 32:lane * 32 + 1, :SP])

        # ---------- main sampling loop ----------
        # u is (SP, R, T): each partition (sample) owns a contiguous
        # R*T*4 = 100KB DRAM run. Stream CH r's per chunk so every DMA
        # moves CH*2KB contiguous per partition (large packets), compute
        # Ln twice in-place chunk-wide, then per r-pair: PE-broadcast the
        # logits rows into PSUM, subtract, exp (+row-sum), normalize.
        for r0 in range(0, R, CH):
            if r0 in pre_ut:
                ut = pre_ut[r0]
            else:
                ut = upool.tile([SP, CW], F32, tag="u", name="u")
                nc.sync.dma_start(
                    ut[:].rearrange("p (c t) -> p c t", c=CH),
                    din["u"][:, r0:r0 + CH, :])
            # a = exp(-ln(-ln u)) = -1/ln(u), three chunk-wide in-place
            # ACT passes (one table set), all independent of the logits
            nc.scalar.activation(ut[:], ut[:], mybir.ActivationFunctionType.Ln)
            nc.scalar.activation(ut[:], ut[:], mybir.ActivationFunctionType.Ln,
                                 scale=-1.0)
            nc.scalar.activation(ut[:], ut[:], mybir.ActivationFunctionType.Exp,
                                 scale=-1.0)
            ot = opool.tile([SP, CW], F32, tag="o", name="o")
            for g in range(CH):
                seg = slice(g * T, (g + 1) * T)
                # broadcast L row r across partitions via ones-matmuls
                # (hi+lo bf16 planes accumulate in f32 PSUM)
                rhs_hi, rhs_lo, lhs_ones = lg_slice(r0 + g)
                bt = bppool.tile([SP, 512], F32, tag="bp", name="bp")
                nc.tensor.matmul(bt[:, :T], lhs_ones, rhs_hi,
                                 start=True, stop=False)
                nc.tensor.matmul(bt[:, :T], lhs_ones, rhs_lo,
                                 start=False, stop=True)
                # q = a * L_bcast with fused row-sum
                # (tensor_tensor_reduce fails NEFF-side on this stack;
                # scalar_tensor_tensor with op0=bypass is HW-proven).
                # q goes to a per-r tile, not an ot slice: in-place
                # chains on one chunk tile serialize all 5 r's.
                qt = qpool.tile([SP, T], F32, tag="q", name="q")
                ss = spool.tile([SP, 1], F32, tag="ss", name="ss")
                nc.vector.scalar_tensor_tensor(
                    qt[:], bt[:, :T], 0.0, ut[:, seg],
                    op0=mybir.AluOpType.bypass, op1=mybir.AluOpType.mult,
                    accum_out=ss[:])
                rs = spool.tile([SP, 1], F32, tag="rs", name="rs")
                nc.vector.reciprocal(rs[:], ss[:])
                nc.vector.tensor_scalar_mul(ot[:, seg], qt[:], rs[:])
            nc.gpsimd.dma_start(out[:, r0:r0 + CH, :],
                                ot[:].rearrange("p (c t) -> p c t", c=CH))


def _get_nc():
    if Z_MODE not in _CACHE:
        _CACHE[Z_MODE] = _build(Z_MODE)
    return _CACHE[Z_MODE]


def prep_in_maps(inputs):
    f32 = np.float32
    state = np.asarray(inputs["state"], f32)[0]          # (500, 2)
    payoff = np.asarray(inputs["payoff"], f32)           # (500, 500)
    noise = np.asarray(inputs["feat_noise"], f32)[0]     # (500, 2)
    xT = np.concatenate([state, payoff, noise], axis=1).T.copy()  # (504, 500)
    gamma = np.asarray(inputs["bn_gamma"], f32)
    beta = np.asarray(inputs["bn_beta"], f32)
    adjT = (np.asarray(inputs["norm_adj"], f32) * gamma[:, None]).T.copy()
    dclT = np.asarray(inputs["def_cur_loc"], f32).T.copy()
    wr_full = np.asarray(inputs["actgen_w"], f32).reshape(T, H2, T)
    wr_full = np.ascontiguousarray(wr_full.transpose(1, 0, 2))  # (16, 500, 500)
    common = {
        "xT": xT,
        "adjT": adjT,
        "w1": np.asarray(inputs["gc1_w"], f32),
        "b1": np.asarray(inputs["gc1_b"], f32).reshape(1, H1),
        "w2": np.asarray(inputs["gc2_w"], f32),
        "b2": np.asarray(inputs["gc2_b"], f32).reshape(1, H2),
        "grow": gamma.reshape(1, T).copy(),
        "brow": beta.reshape(1, T).copy(),
        "dclT": dclT,
        "av": np.asarray(inputs["actgen_v"], f32),
        "ident": np.eye(128, dtype=f32),
    }
    u = np.asarray(inputs["gumbel_u"], f32)              # (1000, 50, 500)
    w2 = np.asarray(inputs["gc2_w"], f32)
    b2 = np.asarray(inputs["gc2_b"], f32)
    if Z_MODE != "allreduce":
        import ml_dtypes
        wr_bf16 = wr_full.astype(ml_dtypes.bfloat16)
    in_maps = []
    for i in range(NCORES):
        m = dict(common)
        if Z_MODE == "allreduce":
            # SPMD cores index h2[:, 0:2] for their z-shard matmuls, so
            # permute the gc2 output channels per core to put the owned
            # channels (2i, 2i+1) at local 0,1. Channels only feed z, so
            # the permutation changes nothing else.
            perm = [2 * i, 2 * i + 1] + [c for c in range(H2)
                                         if c not in (2 * i, 2 * i + 1)]
            m["w2"] = np.ascontiguousarray(w2[:, perm])
            m["b2"] = np.ascontiguousarray(b2[perm]).reshape(1, H2)
            m["wr"] = np.ascontiguousarray(wr_full[2 * i:2 * i + 2])
        else:
            m["wr"] = wr_bf16
        m["u"] = np.ascontiguousarray(u[i * SP:(i + 1) * SP])  # (125, 50, 500)
        in_maps.append(m)
    return in_maps


def run(inputs, trace=False):
    nc = _get_nc()
    in_maps = prep_in_maps(inputs)
    res = bass_utils.run_bass_kernel_spmd(
        nc, in_maps, core_ids=list(range(NCORES)), trace=trace)
    full = np.concatenate([res.results[i]["out"] for i in range(NCORES)],
                          axis=0)                        # (1000, 50, 500)
    return full, res


def kernel(**inputs):
    full, _ = run(inputs)
    return full



# revision 3
# speedup vs baseline: 1.1668x; 1.1668x over previous
"""Trainium2 Bass kernel for nn_Def_A2C_Sample_Generator.

Computation (see reference):
  x = concat(state, payoff, noise)            (500, 504)
  h1 = lrelu(bn(adj @ (x @ w1) + b1))         (500, 32)
  h2 = lrelu(bn(adj @ (h1 @ w2) + b2))        (500, 16)
  xf = h2.reshape(8000)
  logits = xf @ actgen_w + def_cur_loc @ actgen_v          (50, 500)
  out = softmax(logits[None] + gumbel(u), axis=-1)         (1000, 50, 500)

Sharding: data-parallel over the 1000 samples, 125 per core on 8
cores. Each core computes the logits redundantly (small GCN; the 8 MB
bf16 actgen_w is streamed) and softmaxes its own 125 x 50 x 500 gumbel
block.

Softmax is factored so every ACT pass is independent of the logits
(the logits path is the serial prologue; all gumbel work front-runs
it):
  exp(logits + g) with g = -ln(-ln u) equals L * a where
  L = exp(logits) (prologue, 50x500) and a = exp(-ln(-ln u)) = -1/ln u.
Main loop, 5-r chunks in the natural (sample, r, T) layout:
  a       : 3 chunk-wide in-place ACT passes (Ln, Ln(-x), Exp(-x);
            one table set - see the act-table monkeypatch below)
  L bcast : per-r PE ones-matmul, single bf16 plane into f32 PSUM
  q, S    : DVE scalar_tensor_tensor mult with fused row-sum accum
  out     : one DVE reciprocal per chunk + per-r tensor_scalar mult
            into a bf16 chunk tile, one 625KB store per chunk
            (host upcasts to f32; bf16 rounding is ~2e-3 rel, the
            harness gate is 2e-2)

TRN2 PE runs at a ~5x-slow mid p-state unless continuously busy for
3us, and f32 matmuls cost 4 cyc/row on top -- so every prologue
matmul operand that tolerates bf16 (adjT, av, dclT, bias rows, zrow,
xw tiles) is bf16, and the per-r broadcast is one bf16 plane (~0.2%
rounding on L, well inside the error budget).

DMA queues: u loads + params on the sync HWDGE ring (NOT the scalar
ring: HWDGE descriptor generation executes on the issuing engine, and
the scalar ring would burn ~40us of ACT engine time), actgen_w stream
+ output stores on the gpsimd SWDGE path. actgen_w is host-packed
per-partition-contiguous ([16,125,2000]) so each channel load is 125
4KB-run descriptors instead of 500 1KB ones.

Logits rows are packed into 3 lanes at base partitions 0/32/64 (the
only legal matmul operand bases) via a DRAM bounce.
"""
import sys

if "/opt/trn_rl_repo" not in sys.path:
    sys.path.insert(0, "/opt/trn_rl_repo")

import numpy as np

import concourse.bacc as bacc
import concourse.bass as bass
import concourse.mybir as mybir
import concourse.tile as tile
from concourse import bass_utils

# The act-table-load pass resolves Exp -> exp_and_others (id 0) and
# Ln -> natural_log (id 5), thrashing a ~2.7us table swap at every
# Ln<->Exp transition in the main loop. natural_log_exp_and_others
# (id 6) holds BOTH. Hide exp/ln from the other sets in the map the
# chooser reads (ids keep indexing the real act_info.json, so the
# loaded tables are unchanged) so every Exp and Ln lands on set 6 and
# one load suffices.
_orig_get_act_tables = bacc.get_activation_tables


def _patched_get_act_tables(arch):
    tabs = dict(_orig_get_act_tables(arch))
    both = {mybir.ActivationFunctionType.Exp, mybir.ActivationFunctionType.Ln}
    for name, fns in tabs.items():
        if name != "natural_log_exp_and_others" and (both & fns):
            tabs[name] = fns - both
    return tabs


bacc.get_activation_tables = _patched_get_act_tables

F32 = mybir.dt.float32
BF16 = mybir.dt.bfloat16
NCORES = 8
T = 500
R = 50
NS = 1000
SP = NS // NCORES  # 125 samples per core
H1, H2 = 32, 16
FIN = 504  # 2 + 500 + 2 input features
KT = 4  # K/M tiling of the 500 dim into 4x125
NEG_SLOPE = 0.2

_CACHE = {}


def _build():
    nc = bacc.Bacc("TRN2", target_bir_lowering=False, debug=False,
                   enable_asserts=False, num_devices=NCORES)

    # ---- I/O ----
    din = {}
    din["xT"] = nc.dram_tensor("xT", [FIN, T], F32, kind="ExternalInput")
    din["adjT"] = nc.dram_tensor("adjT", [T, T], BF16, kind="ExternalInput")
    din["w1"] = nc.dram_tensor("w1", [FIN, H1], F32, kind="ExternalInput")
    din["b1"] = nc.dram_tensor("b1", [1, H1], BF16, kind="ExternalInput")
    din["w2"] = nc.dram_tensor("w2", [H1, H2], F32, kind="ExternalInput")
    din["b2"] = nc.dram_tensor("b2", [1, H2], BF16, kind="ExternalInput")
    din["grow"] = nc.dram_tensor("grow", [1, T], BF16, kind="ExternalInput")
    din["brow"] = nc.dram_tensor("brow", [1, T], BF16, kind="ExternalInput")
    din["dclT"] = nc.dram_tensor("dclT", [T, R], BF16, kind="ExternalInput")
    din["av"] = nc.dram_tensor("av", [T, T], BF16, kind="ExternalInput")
    # host-packed [c, p, k*T]: partition-contiguous channel planes
    din["wr"] = nc.dram_tensor("wr", [H2, 125, KT * T], BF16,
                               kind="ExternalInput")
    din["ident"] = nc.dram_tensor("ident", [128, 128], F32, kind="ExternalInput")
    din["u"] = nc.dram_tensor("u", [SP, R, T], F32, kind="ExternalInput")
    out = nc.dram_tensor("out", [SP, R, T], BF16, kind="ExternalOutput")

    with tile.TileContext(nc) as tc:
        _emit(nc, tc, din, out)
    nc.compile()
    return nc


def _emit(nc, tc, din, out):
    from contextlib import ExitStack

    ctx = ExitStack()
    with ctx:
        # ---------- pools ----------
        const = ctx.enter_context(tc.tile_pool(name="const", bufs=1))
        small = ctx.enter_context(tc.tile_pool(name="small", bufs=1))
        psum = ctx.enter_context(tc.tile_pool(name="psum", bufs=1, space="PSUM"))
        dram = ctx.enter_context(tc.tile_pool(name="dram", bufs=1, space="DRAM"))

        # ---------- main-loop pools (created first: the first u-chunk
        # loads go ahead of the params in the sync FIFO so the ACT
        # front-run starts immediately) ----------
        CH = 5  # r's per chunk
        CW = CH * T
        upool = ctx.enter_context(tc.tile_pool(name="upool", bufs=5))
        opool = ctx.enter_context(tc.tile_pool(name="opool", bufs=3))
        qpool = ctx.enter_context(tc.tile_pool(name="qpool", bufs=6))
        spool = ctx.enter_context(tc.tile_pool(name="spool", bufs=4))
        bppool = ctx.enter_context(tc.tile_pool(name="bppool", bufs=5,
                                                space="PSUM"))
        pre_ut = {}
        for r0 in (0, CH):
            ut = upool.tile([SP, CW], F32, tag="u", name="u")
            nc.sync.dma_start(
                ut[:].rearrange("p (c t) -> p c t", c=CH),
                din["u"][:, r0:r0 + CH, :])
            pre_ut[r0] = ut

        # ---------- load params ----------
        ident = const.tile([128, 128], F32, tag="ident", name="ident")
        nc.sync.dma_start(ident[:], din["ident"][:])
        onesb = const.tile([65, 128], BF16, tag="onesb", name="onesb")
        nc.vector.memset(onesb[:], 1.0)

        xT = [const.tile([126, T], F32, tag=f"xT{k}", name=f"xT{k}") for k in range(KT)]
        for k in range(KT):
            nc.sync.dma_start(xT[k][:], din["xT"][k * 126:(k + 1) * 126, :])
        adjT = [const.tile([125, T], BF16, tag=f"adjT{k}", name=f"adjT{k}") for k in range(KT)]
        for k in range(KT):
            nc.sync.dma_start(adjT[k][:], din["adjT"][k * 125:(k + 1) * 125, :])
        w1 = [const.tile([126, H1], F32, tag=f"w1{k}", name=f"w1{k}") for k in range(KT)]
        for k in range(KT):
            nc.sync.dma_start(w1[k][:], din["w1"][k * 126:(k + 1) * 126, :])
        b1 = const.tile([1, H1], BF16, tag="b1", name="b1")
        nc.sync.dma_start(b1[:], din["b1"][:])
        w2 = const.tile([H1, H2], F32, tag="w2", name="w2")
        nc.sync.dma_start(w2[:], din["w2"][:])
        b2 = const.tile([1, H2], BF16, tag="b2", name="b2")
        nc.sync.dma_start(b2[:], din["b2"][:])
        grow = const.tile([1, T], BF16, tag="grow", name="grow")
        nc.sync.dma_start(grow[:], din["grow"][:])
        brow = const.tile([1, T], BF16, tag="brow", name="brow")
        nc.sync.dma_start(brow[:], din["brow"][:])
        dclT = [const.tile([125, R], BF16, tag=f"dclT{k}", name=f"dclT{k}") for k in range(KT)]
        for k in range(KT):
            nc.sync.dma_start(dclT[k][:], din["dclT"][k * 125:(k + 1) * 125, :])
        av = [const.tile([125, T], BF16, tag=f"av{k}", name=f"av{k}") for k in range(KT)]
        for k in range(KT):
            nc.sync.dma_start(av[k][:], din["av"][k * 125:(k + 1) * 125, :])

        # ---------- GCN, transposed formulation ----------
        # bn is folded into the adjacency on the host (adjT ships
        # gamma[t]*adj[t,u] transposed), leaving rank-1 bias terms:
        #   bn(adj@xw+b)^T[c,t] = (xw^T adj1^T)[c,t] + b[c]*gamma[t]
        #                         + beta[t]
        # so each adj product is ONE [H,500] PSUM accumulation of 4
        # K-tiles plus two K=1 bias matmuls, and layer 2 consumes h1T
        # directly as its stationary operand (no transposes, no bn DVE
        # chain).
        def lrelu_from_psum(ps_ap, out_tile, width):
            tmp = small.tile([width, T], F32, tag=f"lr{width}", name=f"lr{width}")
            nc.vector.tensor_scalar_mul(tmp[:], ps_ap, NEG_SLOPE)
            nc.vector.tensor_tensor(out_tile[:], tmp[:], ps_ap,
                                    op=mybir.AluOpType.max)

        xw1 = [small.tile([125, H1], BF16, tag=f"xw1{m}", name=f"xw1{m}") for m in range(KT)]
        for m in range(KT):
            ps = psum.tile([125, H1], F32, tag="ps_small", name="ps_small")
            for k in range(KT):
                nc.tensor.matmul(ps[:], xT[k][:, m * 125:(m + 1) * 125],
                                 w1[k][:], start=(k == 0), stop=(k == KT - 1))
            nc.vector.tensor_copy(xw1[m][:], ps[:])

        a1ps = psum.tile([H1, T], F32, tag="ps_small", name="ps_small")
        for k in range(KT):
            nc.tensor.matmul(a1ps[:], xw1[k][:], adjT[k][:],
                             start=(k == 0), stop=False)
        nc.tensor.matmul(a1ps[:], b1[:], grow[:], start=False, stop=False)
        nc.tensor.matmul(a1ps[:], onesb[0:1, :H1], brow[:], start=False,
                         stop=True)
        h1T = small.tile([H1, T], F32, tag="h1T", name="h1T")
        lrelu_from_psum(a1ps[:], h1T, H1)

        xw2 = [small.tile([125, H2], BF16, tag=f"xw2{m}", name=f"xw2{m}") for m in range(KT)]
        for m in range(KT):
            ps = psum.tile([125, H2], F32, tag="ps_small", name="ps_small")
            nc.tensor.matmul(ps[:], h1T[:, m * 125:(m + 1) * 125], w2[:],
                             start=True, stop=True)
            nc.vector.tensor_copy(xw2[m][:], ps[:])

        a2ps = psum.tile([H2, T], F32, tag="ps_small", name="ps_small")
        for k in range(KT):
            nc.tensor.matmul(a2ps[:], xw2[k][:], adjT[k][:],
                             start=(k == 0), stop=False)
        nc.tensor.matmul(a2ps[:], b2[:], grow[:], start=False, stop=False)
        nc.tensor.matmul(a2ps[:], onesb[0:1, :H2], brow[:], start=False,
                         stop=True)
        h2T = small.tile([H2, T], F32, tag="h2T", name="h2T")
        lrelu_from_psum(a2ps[:], h2T, H2)

        # h2 back to [t, c] tiles in bf16 for the z matmuls
        h2b = [small.tile([125, H2], BF16, tag=f"h2b{k}", name=f"h2b{k}")
               for k in range(KT)]
        for k in range(KT):
            pt = psum.tile([125, H2], F32, tag="ps_small", name="ps_small")
            nc.tensor.transpose(pt[:], h2T[:, k * 125:(k + 1) * 125],
                                ident[:H2, :H2])
            nc.vector.tensor_copy(h2b[k][:], pt[:])

        # ---------- z = xf @ actgen_w (bf16 stream; f32 PSUM accum) ----------
        zps = psum.tile([1, T], F32, tag="ps_z", name="ps_z")
        wpool = ctx.enter_context(tc.tile_pool(name="wpool", bufs=9))
        first = True
        for c in range(H2):
            wt = wpool.tile([125, KT * T], BF16, tag="wr_stream",
                            name="wr_stream")
            nc.gpsimd.dma_start(wt[:], din["wr"][c])
            for k in range(KT):
                nc.tensor.matmul(zps[:], h2b[k][:, c:c + 1],
                                 wt[:, k * T:(k + 1) * T],
                                 start=first,
                                 stop=(c == H2 - 1 and k == KT - 1))
                first = False
        zrow = small.tile([1, T], BF16, tag="zrow", name="zrow")
        nc.vector.tensor_copy(zrow[:], zps[:])

        # ---------- logits = dcl @ av + z (broadcast over rows) ----------
        lgp = psum.tile([R, T], F32, tag="ps_lg", name="ps_lg")
        for k in range(KT):
            nc.tensor.matmul(lgp[:], dclT[k][:], av[k][:],
                             start=(k == 0), stop=False)
        nc.tensor.matmul(lgp[:], onesb[0:1, :R], zrow[:], start=False, stop=True)
        # matmul operands need base partition in {0, 32, 64}; pack the 50
        # L = exp(logits) rows (bf16) into 3 lanes at those partitions,
        # 17 rows each along the free dim. Bounce through DRAM to reshape
        # partitions->free in 3 DMAs.
        LPL = 17  # logits rows per lane
        lgb = small.tile([R, T], BF16, tag="lgb", name="lgb")
        nc.scalar.activation(lgb[:], lgp[:],
                             mybir.ActivationFunctionType.Exp)
        ld = dram.tile([R, T], BF16, name="ldram")
        nc.sync.dma_start(ld[:], lgb[:])
        fl = small.tile([65, LPL * T], BF16, tag="lgflat", name="lgflat")
        nc.sync.dma_start(
            fl[0:33:32, :].rearrange("l (j t) -> l j t", j=LPL),
            ld[0:2 * LPL].rearrange("(l j) t -> l j t", l=2))
        nc.sync.dma_start(fl[64:65, :(R - 2 * LPL) * T],
                          ld[2 * LPL:R].rearrange("(o j) t -> o (j t)", o=1))

        def lg_slice(r):
            lane, j = r // LPL, r % LPL
            return (fl[lane * 32:lane * 32 + 1, j * T:(j + 1) * T],
                    onesb[lane * 32:lane * 32 + 1, :SP])

        # ---------- main sampling loop ----------
        # u is (SP, R, T): each partition (sample) owns a contiguous
        # R*T*4 = 100KB DRAM run. Stream CH r's per chunk so every DMA
        # moves CH*2KB contiguous per partition (large packets), compute
        # a = -1/ln u in 3 chunk-wide in-place ACT passes, then per r:
        # PE-broadcast the L row into PSUM, multiply (+row-sum), then
        # one chunk-wide reciprocal and per-r normalize into bf16.
        for r0 in range(0, R, CH):
            if r0 in pre_ut:
                ut = pre_ut[r0]
            else:
                ut = upool.tile([SP, CW], F32, tag="u", name="u")
                nc.sync.dma_start(
                    ut[:].rearrange("p (c t) -> p c t", c=CH),
                    din["u"][:, r0:r0 + CH, :])
            # a = exp(-ln(-ln u)) = -1/ln(u), three chunk-wide in-place
            # ACT passes (one table set), all independent of the logits
            nc.scalar.activation(ut[:], ut[:], mybir.ActivationFunctionType.Ln)
            nc.scalar.activation(ut[:], ut[:], mybir.ActivationFunctionType.Ln,
                                 scale=-1.0)
            nc.scalar.activation(ut[:], ut[:], mybir.ActivationFunctionType.Exp,
                                 scale=-1.0)
            ot = opool.tile([SP, CW], BF16, tag="o", name="o")
            ssc = spool.tile([SP, CH], F32, tag="ss", name="ss")
            rsc = spool.tile([SP, CH], F32, tag="rs", name="rs")
            qts = []
            for g in range(CH):
                seg = slice(g * T, (g + 1) * T)
                # broadcast L row r across partitions via a ones-matmul
                rhs, lhs_ones = lg_slice(r0 + g)
                bt = bppool.tile([SP, 512], F32, tag="bp", name="bp")
                nc.tensor.matmul(bt[:, :T], lhs_ones, rhs,
                                 start=True, stop=True)
                # q = a * L_bcast with fused row-sum
                # (tensor_tensor_reduce fails NEFF-side on this stack;
                # scalar_tensor_tensor with op0=bypass is HW-proven).
                # q goes to a per-r tile, not an ot slice: in-place
                # chains on one chunk tile serialize all 5 r's.
                qt = qpool.tile([SP, T], F32, tag="q", name="q")
                nc.vector.scalar_tensor_tensor(
                    qt[:], bt[:, :T], 0.0, ut[:, seg],
                    op0=mybir.AluOpType.bypass, op1=mybir.AluOpType.mult,
                    accum_out=ssc[:, g:g + 1])
                qts.append(qt)
            nc.vector.reciprocal(rsc[:], ssc[:])
            for g in range(CH):
                seg = slice(g * T, (g + 1) * T)
                nc.vector.tensor_scalar_mul(ot[:, seg], qts[g][:],
                                            rsc[:, g:g + 1])
            nc.gpsimd.dma_start(out[:, r0:r0 + CH, :],
                                ot[:].rearrange("p (c t) -> p c t", c=CH))


def _get_nc():
    if "nc" not in _CACHE:
        _CACHE["nc"] = _build()
    return _CACHE["nc"]


def prep_in_maps(inputs):
    import ml_dtypes
    f32 = np.float32
    bf16 = ml_dtypes.bfloat16
    state = np.asarray(inputs["state"], f32)[0]          # (500, 2)
    payoff = np.asarray(inputs["payoff"], f32)           # (500, 500)
    noise = np.asarray(inputs["feat_noise"], f32)[0]     # (500, 2)
    xT = np.concatenate([state, payoff, noise], axis=1).T.copy()  # (504, 500)
    gamma = np.asarray(inputs["bn_gamma"], f32)
    beta = np.asarray(inputs["bn_beta"], f32)
    adjT = (np.asarray(inputs["norm_adj"], f32) * gamma[:, None]).T
    dclT = np.asarray(inputs["def_cur_loc"], f32).T
    wr_full = np.asarray(inputs["actgen_w"], f32).reshape(T, H2, T)
    wr_full = wr_full.transpose(1, 0, 2)                 # (16, 500, 500)
    # partition-contiguous channel planes: [c, p, k*T] with
    # wr_pack[c, p, k*T + t] = wr_full[c, k*125 + p, t]
    wr_pack = np.ascontiguousarray(
        wr_full.reshape(H2, KT, 125, T).transpose(0, 2, 1, 3)
    ).reshape(H2, 125, KT * T).astype(bf16)
    common = {
        "xT": xT,
        "adjT": np.ascontiguousarray(adjT).astype(bf16),
        "w1": np.asarray(inputs["gc1_w"], f32),
        "b1": np.asarray(inputs["gc1_b"], f32).reshape(1, H1).astype(bf16),
        "w2": np.asarray(inputs["gc2_w"], f32),
        "b2": np.asarray(inputs["gc2_b"], f32).reshape(1, H2).astype(bf16),
        "grow": gamma.reshape(1, T).astype(bf16),
        "brow": beta.reshape(1, T).astype(bf16),
        "dclT": np.ascontiguousarray(dclT).astype(bf16),
        "av": np.asarray(inputs["actgen_v"], f32).astype(bf16),
        "wr": wr_pack,
        "ident": np.eye(128, dtype=f32),
    }
    u = np.asarray(inputs["gumbel_u"], f32)              # (1000, 50, 500)
    in_maps = []
    for i in range(NCORES):
        m = dict(common)
        m["u"] = np.ascontiguousarray(u[i * SP:(i + 1) * SP])  # (125, 50, 500)
        in_maps.append(m)
    return in_maps


def run(inputs, trace=False):
    nc = _get_nc()
    in_maps = prep_in_maps(inputs)
    res = bass_utils.run_bass_kernel_spmd(
        nc, in_maps, core_ids=list(range(NCORES)), trace=trace)
    full = np.concatenate(
        [np.asarray(res.results[i]["out"]).astype(np.float32)
         for i in range(NCORES)], axis=0)                # (1000, 50, 500)
    return full, res


def kernel(**inputs):
    full, _ = run(inputs)
    return full


# revision 5
# speedup vs baseline: 1.1929x; 1.0224x over previous
"""Trainium2 Bass kernel for nn_Def_A2C_Sample_Generator.

Computation (see reference):
  x = concat(state, payoff, noise)            (500, 504)
  h1 = lrelu(bn(adj @ (x @ w1) + b1))         (500, 32)
  h2 = lrelu(bn(adj @ (h1 @ w2) + b2))        (500, 16)
  xf = h2.reshape(8000)
  logits = xf @ actgen_w + def_cur_loc @ actgen_v          (50, 500)
  out = softmax(logits[None] + gumbel(u), axis=-1)         (1000, 50, 500)

Sharding: data-parallel over the 1000 samples, 125 per core on 8
cores. Each core computes the logits redundantly (small GCN; the 8 MB
bf16 actgen_w is streamed) and softmaxes its own 125 x 50 x 500 gumbel
block.

Softmax is factored so every ACT pass is independent of the logits
(the logits path is the serial prologue; all gumbel work front-runs
it):
  exp(logits + g) with g = -ln(-ln u) equals L * a where
  L = exp(logits) (prologue, 50x500) and a = exp(-ln(-ln u)) = -1/ln u.
Main loop, 5-r chunks in the natural (sample, r, T) layout:
  a       : 3 chunk-wide in-place ACT passes (Ln, Ln(-x), Exp(-x);
            one table set - see the act-table monkeypatch below)
  L bcast : per-r PE ones-matmul, single bf16 plane into f32 PSUM
  q, S    : DVE scalar_tensor_tensor mult with fused row-sum accum
  out     : one DVE reciprocal per chunk + per-r tensor_scalar mult
            into a bf16 chunk tile, one 625KB store per chunk
            (host upcasts to f32; bf16 rounding is ~2e-3 rel, the
            harness gate is 2e-2)

TRN2 PE runs at a ~5x-slow mid p-state unless continuously busy for
3us, and f32 matmuls cost 4 cyc/row on top -- so every prologue
matmul operand that tolerates bf16 (adjT, av, dclT, bias rows, zrow,
xw tiles) is bf16, and the per-r broadcast is one bf16 plane (~0.2%
rounding on L, well inside the error budget).

DMA queues: u loads + params on the sync HWDGE ring (NOT the scalar
ring: HWDGE descriptor generation executes on the issuing engine, and
the scalar ring would burn ~40us of ACT engine time), actgen_w stream
+ output stores on the gpsimd SWDGE path. actgen_w is host-packed
per-partition-contiguous ([16,125,2000]) so each channel load is 125
4KB-run descriptors instead of 500 1KB ones.

Logits rows are packed into 3 lanes at base partitions 0/32/64 (the
only legal matmul operand bases) via a DRAM bounce.
"""
import sys

if "/opt/trn_rl_repo" not in sys.path:
    sys.path.insert(0, "/opt/trn_rl_repo")

import numpy as np

import concourse.bacc as bacc
import concourse.bass as bass
import concourse.mybir as mybir
import concourse.tile as tile
from concourse import bass_utils

# The act-table-load pass resolves Exp -> exp_and_others (id 0) and
# Ln -> natural_log (id 5), thrashing a ~2.7us table swap at every
# Ln<->Exp transition in the main loop. natural_log_exp_and_others
# (id 6) holds BOTH. Hide exp/ln from the other sets in the map the
# chooser reads (ids keep indexing the real act_info.json, so the
# loaded tables are unchanged) so every Exp and Ln lands on set 6 and
# one load suffices.
_orig_get_act_tables = bacc.get_activation_tables


def _patched_get_act_tables(arch):
    tabs = dict(_orig_get_act_tables(arch))
    both = {mybir.ActivationFunctionType.Exp, mybir.ActivationFunctionType.Ln}
    for name, fns in tabs.items():
        if name != "natural_log_exp_and_others" and (both & fns):
            tabs[name] = fns - both
    return tabs


bacc.get_activation_tables = _patched_get_act_tables

F32 = mybir.dt.float32
BF16 = mybir.dt.bfloat16
NCORES = 8
T = 500
R = 50
NS = 1000
SP = NS // NCORES  # 125 samples per core
H1, H2 = 32, 16
FIN = 504  # 2 + 500 + 2 input features
KT = 4  # K/M tiling of the 500 dim into 4x125
NEG_SLOPE = 0.2

_CACHE = {}


def _build():
    nc = bacc.Bacc("TRN2", target_bir_lowering=False, debug=False,
                   enable_asserts=False, num_devices=NCORES)

    # ---- I/O ----
    din = {}
    din["xT"] = nc.dram_tensor("xT", [FIN, T], F32, kind="ExternalInput")
    din["adjT"] = nc.dram_tensor("adjT", [T, T], BF16, kind="ExternalInput")
    din["w1"] = nc.dram_tensor("w1", [FIN, H1], F32, kind="ExternalInput")
    din["b1"] = nc.dram_tensor("b1", [1, H1], BF16, kind="ExternalInput")
    din["w2"] = nc.dram_tensor("w2", [H1, H2], F32, kind="ExternalInput")
    din["b2"] = nc.dram_tensor("b2", [1, H2], BF16, kind="ExternalInput")
    din["grow"] = nc.dram_tensor("grow", [1, T], BF16, kind="ExternalInput")
    din["brow"] = nc.dram_tensor("brow", [1, T], BF16, kind="ExternalInput")
    din["dclT"] = nc.dram_tensor("dclT", [T, R], BF16, kind="ExternalInput")
    din["av"] = nc.dram_tensor("av", [T, T], BF16, kind="ExternalInput")
    # host-packed [c, p, k*T]: partition-contiguous channel planes
    din["wr"] = nc.dram_tensor("wr", [H2, 125, KT * T], BF16,
                               kind="ExternalInput")
    din["ident"] = nc.dram_tensor("ident", [128, 128], F32, kind="ExternalInput")
    din["u"] = nc.dram_tensor("u", [SP, R, T], F32, kind="ExternalInput")
    out = nc.dram_tensor("out", [SP, R, T], BF16, kind="ExternalOutput")

    with tile.TileContext(nc) as tc:
        _emit(nc, tc, din, out)
    nc.compile()
    return nc


def _emit(nc, tc, din, out):
    from contextlib import ExitStack

    ctx = ExitStack()
    with ctx:
        # ---------- pools ----------
        const = ctx.enter_context(tc.tile_pool(name="const", bufs=1))
        small = ctx.enter_context(tc.tile_pool(name="small", bufs=1))
        psum = ctx.enter_context(tc.tile_pool(name="psum", bufs=1, space="PSUM"))
        dram = ctx.enter_context(tc.tile_pool(name="dram", bufs=1, space="DRAM"))

        # ---------- pools for the main loop ----------
        CH = 5  # r's per chunk
        CW = CH * T
        upool = ctx.enter_context(tc.tile_pool(name="upool", bufs=5))
        opool = ctx.enter_context(tc.tile_pool(name="opool", bufs=3))
        qpool = ctx.enter_context(tc.tile_pool(name="qpool", bufs=6))
        spool = ctx.enter_context(tc.tile_pool(name="spool", bufs=4))
        bppool = ctx.enter_context(tc.tile_pool(name="bppool", bufs=5,
                                                space="PSUM"))

        # ---------- load params (HEAD of the sync FIFO: the HWDGE ring
        # drains in emission order, so the GCN/logits critical path must
        # come before the 12.5MB u stream) ----------
        onesb = const.tile([65, 128], BF16, tag="onesb", name="onesb")
        nc.vector.memset(onesb[:], 1.0)

        xT = [const.tile([126, T], F32, tag=f"xT{k}", name=f"xT{k}") for k in range(KT)]
        w1 = [const.tile([126, H1], F32, tag=f"w1{k}", name=f"w1{k}") for k in range(KT)]
        adjT = [const.tile([125, T], BF16, tag=f"adjT{k}", name=f"adjT{k}") for k in range(KT)]
        for k in range(KT):
            nc.sync.dma_start(w1[k][:], din["w1"][k * 126:(k + 1) * 126, :])
        for k in range(KT):
            nc.sync.dma_start(xT[k][:], din["xT"][k * 126:(k + 1) * 126, :])
        for k in range(KT):
            nc.sync.dma_start(adjT[k][:], din["adjT"][k * 125:(k + 1) * 125, :])
        b1 = const.tile([1, H1], BF16, tag="b1", name="b1")
        nc.sync.dma_start(b1[:], din["b1"][:])
        w2 = const.tile([H1, H2], F32, tag="w2", name="w2")
        nc.sync.dma_start(w2[:], din["w2"][:])
        b2 = const.tile([1, H2], BF16, tag="b2", name="b2")
        nc.sync.dma_start(b2[:], din["b2"][:])
        grow = const.tile([1, T], BF16, tag="grow", name="grow")
        nc.sync.dma_start(grow[:], din["grow"][:])
        brow = const.tile([1, T], BF16, tag="brow", name="brow")
        nc.sync.dma_start(brow[:], din["brow"][:])
        dclT = [const.tile([125, R], BF16, tag=f"dclT{k}", name=f"dclT{k}") for k in range(KT)]
        for k in range(KT):
            nc.sync.dma_start(dclT[k][:], din["dclT"][k * 125:(k + 1) * 125, :])
        av = [const.tile([125, T], BF16, tag=f"av{k}", name=f"av{k}") for k in range(KT)]
        for k in range(KT):
            nc.sync.dma_start(av[k][:], din["av"][k * 125:(k + 1) * 125, :])
        ident = const.tile([128, 128], F32, tag="ident", name="ident")
        nc.sync.dma_start(ident[:], din["ident"][:])

        # first two u chunks, behind the params but ahead of the rest
        pre_ut = {}
        for r0 in (0, CH):
            ut = upool.tile([SP, CW], F32, tag="u", name="u")
            nc.sync.dma_start(
                ut[:].rearrange("p (c t) -> p c t", c=CH),
                din["u"][:, r0:r0 + CH, :])
            pre_ut[r0] = ut

        # ---------- GCN, transposed formulation ----------
        # bn is folded into the adjacency on the host (adjT ships
        # gamma[t]*adj[t,u] transposed), leaving rank-1 bias terms:
        #   bn(adj@xw+b)^T[c,t] = (xw^T adj1^T)[c,t] + b[c]*gamma[t]
        #                         + beta[t]
        # so each adj product is ONE [H,500] PSUM accumulation of 4
        # K-tiles plus two K=1 bias matmuls, and layer 2 consumes h1T
        # directly as its stationary operand (no transposes, no bn DVE
        # chain).
        def lrelu_from_psum(ps_ap, out_tile, width):
            tmp = small.tile([width, T], F32, tag=f"lr{width}", name=f"lr{width}")
            nc.vector.tensor_scalar_mul(tmp[:], ps_ap, NEG_SLOPE)
            nc.vector.tensor_tensor(out_tile[:], tmp[:], ps_ap,
                                    op=mybir.AluOpType.max)

        xw1 = [small.tile([125, H1], BF16, tag=f"xw1{m}", name=f"xw1{m}") for m in range(KT)]
        for m in range(KT):
            ps = psum.tile([125, H1], F32, tag="ps_small", name="ps_small")
            for k in range(KT):
                nc.tensor.matmul(ps[:], xT[k][:, m * 125:(m + 1) * 125],
                                 w1[k][:], start=(k == 0), stop=(k == KT - 1))
            nc.vector.tensor_copy(xw1[m][:], ps[:])

        a1ps = psum.tile([H1, T], F32, tag="ps_small", name="ps_small")
        for k in range(KT):
            nc.tensor.matmul(a1ps[:], xw1[k][:], adjT[k][:],
                             start=(k == 0), stop=False)
        nc.tensor.matmul(a1ps[:], b1[:], grow[:], start=False, stop=False)
        nc.tensor.matmul(a1ps[:], onesb[0:1, :H1], brow[:], start=False,
                         stop=True)
        h1T = small.tile([H1, T], F32, tag="h1T", name="h1T")
        lrelu_from_psum(a1ps[:], h1T, H1)

        xw2 = [small.tile([125, H2], BF16, tag=f"xw2{m}", name=f"xw2{m}") for m in range(KT)]
        for m in range(KT):
            ps = psum.tile([125, H2], F32, tag="ps_small", name="ps_small")
            nc.tensor.matmul(ps[:], h1T[:, m * 125:(m + 1) * 125], w2[:],
                             start=True, stop=True)
            nc.vector.tensor_copy(xw2[m][:], ps[:])

        a2ps = psum.tile([H2, T], F32, tag="ps_small", name="ps_small")
        for k in range(KT):
            nc.tensor.matmul(a2ps[:], xw2[k][:], adjT[k][:],
                             start=(k == 0), stop=False)
        nc.tensor.matmul(a2ps[:], b2[:], grow[:], start=False, stop=False)
        nc.tensor.matmul(a2ps[:], onesb[0:1, :H2], brow[:], start=False,
                         stop=True)
        h2T = small.tile([H2, T], F32, tag="h2T", name="h2T")
        lrelu_from_psum(a2ps[:], h2T, H2)

        # h2 back to [t, c] tiles in bf16 for the z matmuls
        h2b = [small.tile([125, H2], BF16, tag=f"h2b{k}", name=f"h2b{k}")
               for k in range(KT)]
        for k in range(KT):
            pt = psum.tile([125, H2], F32, tag="ps_small", name="ps_small")
            nc.tensor.transpose(pt[:], h2T[:, k * 125:(k + 1) * 125],
                                ident[:H2, :H2])
            nc.vector.tensor_copy(h2b[k][:], pt[:])

        # ---------- z = xf @ actgen_w (bf16 stream; f32 PSUM accum) ----------
        zps = psum.tile([1, T], F32, tag="ps_z", name="ps_z")
        wpool = ctx.enter_context(tc.tile_pool(name="wpool", bufs=9))
        first = True
        for c in range(H2):
            wt = wpool.tile([125, KT * T], BF16, tag="wr_stream",
                            name="wr_stream")
            nc.gpsimd.dma_start(wt[:], din["wr"][c])
            for k in range(KT):
                nc.tensor.matmul(zps[:], h2b[k][:, c:c + 1],
                                 wt[:, k * T:(k + 1) * T],
                                 start=first,
                                 stop=(c == H2 - 1 and k == KT - 1))
                first = False
        zrow = small.tile([1, T], BF16, tag="zrow", name="zrow")
        nc.vector.tensor_copy(zrow[:], zps[:])

        # ---------- logits = dcl @ av + z (broadcast over rows) ----------
        lgp = psum.tile([R, T], F32, tag="ps_lg", name="ps_lg")
        for k in range(KT):
            nc.tensor.matmul(lgp[:], dclT[k][:], av[k][:],
                             start=(k == 0), stop=False)
        nc.tensor.matmul(lgp[:], onesb[0:1, :R], zrow[:], start=False, stop=True)
        # matmul operands need base partition in {0, 32, 64}; pack the 50
        # L = exp(logits) rows (bf16) into 3 lanes at those partitions,
        # 17 rows each along the free dim. Bounce through DRAM to reshape
        # partitions->free in 3 DMAs.
        LPL = 17  # logits rows per lane
        lgb = small.tile([R, T], BF16, tag="lgb", name="lgb")
        nc.scalar.activation(lgb[:], lgp[:],
                             mybir.ActivationFunctionType.Exp)
        # bounce on the scalar ring: the sync ring is busy streaming u
        # chunks and would head-of-line-block these behind them
        ld = dram.tile([R, T], BF16, name="ldram")
        nc.scalar.dma_start(ld[:], lgb[:])
        fl = small.tile([65, LPL * T], BF16, tag="lgflat", name="lgflat")
        nc.scalar.dma_start(
            fl[0:33:32, :].rearrange("l (j t) -> l j t", j=LPL),
            ld[0:2 * LPL].rearrange("(l j) t -> l j t", l=2))
        nc.scalar.dma_start(fl[64:65, :(R - 2 * LPL) * T],
                            ld[2 * LPL:R].rearrange("(o j) t -> o (j t)", o=1))

        def lg_slice(r):
            lane, j = r // LPL, r % LPL
            return (fl[lane * 32:lane * 32 + 1, j * T:(j + 1) * T],
                    onesb[lane * 32:lane * 32 + 1, :SP])

        # ---------- main sampling loop ----------
        # u is (SP, R, T): each partition (sample) owns a contiguous
        # R*T*4 = 100KB DRAM run. Stream CH r's per chunk so every DMA
        # moves CH*2KB contiguous per partition (large packets), compute
        # a = -1/ln u in 3 chunk-wide in-place ACT passes, then per r:
        # PE-broadcast the L row into PSUM, multiply (+row-sum), then
        # one chunk-wide reciprocal and per-r normalize into bf16.
        for r0 in range(0, R, CH):
            if r0 in pre_ut:
                ut = pre_ut[r0]
            else:
                ut = upool.tile([SP, CW], F32, tag="u", name="u")
                nc.sync.dma_start(
                    ut[:].rearrange("p (c t) -> p c t", c=CH),
                    din["u"][:, r0:r0 + CH, :])
            # a = exp(-ln(-ln u)) = -1/ln(u), three chunk-wide in-place
            # ACT passes (one table set), all independent of the logits
            nc.scalar.activation(ut[:], ut[:], mybir.ActivationFunctionType.Ln)
            nc.scalar.activation(ut[:], ut[:], mybir.ActivationFunctionType.Ln,
                                 scale=-1.0)
            nc.scalar.activation(ut[:], ut[:], mybir.ActivationFunctionType.Exp,
                                 scale=-1.0)
            ot = opool.tile([SP, CW], BF16, tag="o", name="o")
            ssc = spool.tile([SP, CH], F32, tag="ss", name="ss")
            rsc = spool.tile([SP, CH], F32, tag="rs", name="rs")
            qts = []
            for g in range(CH):
                seg = slice(g * T, (g + 1) * T)
                # broadcast L row r across partitions via a ones-matmul
                rhs, lhs_ones = lg_slice(r0 + g)
                bt = bppool.tile([SP, 512], F32, tag="bp", name="bp")
                nc.tensor.matmul(bt[:, :T], lhs_ones, rhs,
                                 start=True, stop=True)
                # q = a * L_bcast with fused row-sum
                # (tensor_tensor_reduce fails NEFF-side on this stack;
                # scalar_tensor_tensor with op0=bypass is HW-proven).
                # q goes to a per-r tile, not an ot slice: in-place
                # chains on one chunk tile serialize all 5 r's.
                qt = qpool.tile([SP, T], F32, tag="q", name="q")
                nc.vector.scalar_tensor_tensor(
                    qt[:], bt[:, :T], 0.0, ut[:, seg],
                    op0=mybir.AluOpType.bypass, op1=mybir.AluOpType.mult,
                    accum_out=ssc[:, g:g + 1])
                qts.append(qt)
            nc.vector.reciprocal(rsc[:], ssc[:])
            for g in range(CH):
                seg = slice(g * T, (g + 1) * T)
                nc.vector.tensor_scalar_mul(ot[:, seg], qts[g][:],
                                            rsc[:, g:g + 1])
            nc.gpsimd.dma_start(out[:, r0:r0 + CH, :],
                                ot[:].rearrange("p (c t) -> p c t", c=CH))


def _get_nc():
    if "nc" not in _CACHE:
        _CACHE["nc"] = _build()
    return _CACHE["nc"]


def prep_in_maps(inputs):
    import ml_dtypes
    f32 = np.float32
    bf16 = ml_dtypes.bfloat16
    state = np.asarray(inputs["state"], f32)[0]          # (500, 2)
    payoff = np.asarray(inputs["payoff"], f32)           # (500, 500)
    noise = np.asarray(inputs["feat_noise"], f32)[0]     # (500, 2)
    xT = np.concatenate([state, payoff, noise], axis=1).T.copy()  # (504, 500)
    gamma = np.asarray(inputs["bn_gamma"], f32)
    beta = np.asarray(inputs["bn_beta"], f32)
    adjT = (np.asarray(inputs["norm_adj"], f32) * gamma[:, None]).T
    dclT = np.asarray(inputs["def_cur_loc"], f32).T
    wr_full = np.asarray(inputs["actgen_w"], f32).reshape(T, H2, T)
    wr_full = wr_full.transpose(1, 0, 2)                 # (16, 500, 500)
    # partition-contiguous channel planes: [c, p, k*T] with
    # wr_pack[c, p, k*T + t] = wr_full[c, k*125 + p, t]
    wr_pack = np.ascontiguousarray(
        wr_full.reshape(H2, KT, 125, T).transpose(0, 2, 1, 3)
    ).reshape(H2, 125, KT * T).astype(bf16)
    common = {
        "xT": xT,
        "adjT": np.ascontiguousarray(adjT).astype(bf16),
        "w1": np.asarray(inputs["gc1_w"], f32),
        "b1": np.asarray(inputs["gc1_b"], f32).reshape(1, H1).astype(bf16),
        "w2": np.asarray(inputs["gc2_w"], f32),
        "b2": np.asarray(inputs["gc2_b"], f32).reshape(1, H2).astype(bf16),
        "grow": gamma.reshape(1, T).astype(bf16),
        "brow": beta.reshape(1, T).astype(bf16),
        "dclT": np.ascontiguousarray(dclT).astype(bf16),
        "av": np.asarray(inputs["actgen_v"], f32).astype(bf16),
        "wr": wr_pack,
        "ident": np.eye(128, dtype=f32),
    }
    u = np.asarray(inputs["gumbel_u"], f32)              # (1000, 50, 500)
    in_maps = []
    for i in range(NCORES):
        m = dict(common)
        m["u"] = np.ascontiguousarray(u[i * SP:(i + 1) * SP])  # (125, 50, 500)
        in_maps.append(m)
    return in_maps


def run(inputs, trace=False):
    nc = _get_nc()
    in_maps = prep_in_maps(inputs)
    res = bass_utils.run_bass_kernel_spmd(
        nc, in_maps, core_ids=list(range(NCORES)), trace=trace)
    full = np.concatenate(
        [np.asarray(res.results[i]["out"]).astype(np.float32)
         for i in range(NCORES)], axis=0)                # (1000, 50, 500)
    return full, res


def kernel(**inputs):
    full, _ = run(inputs)
    return full
